# revision 6
# baseline (speedup 1.0000x reference)
"""Multi-head attention (S=2048, B=2, D=1024, H=16) on 8 Trainium2 NeuronCores.

Sharding: batch*head parallel. Core c handles batch b=c//4 and heads
4*(c%4) .. 4*(c%4)+3. Weights are column-sliced (Wq/Wk/Wv) / row-sliced (Wo)
per core; each core produces a partial [S, D] output (Wo row-parallel) and
the host gather sums the 4 partials per batch.

v2: bf16 datapath (inputs/weights/activations; fp32 PSUM accumulate),
single packed weight DMAs, static ones/zeros via gpsimd memset at start,
exp merged across head pairs ([128,1024] psum tiles spanning 2 banks),
reciprocal_approx_fast for the softmax normalize, and out-projection of
chunk c-1 interleaved into chunk c's attention loop (PSUM ring shared
with the score tiles) with merged [128,1024] bf16 output DMAs.

On-device layout (per core):
  qT[dk,s]  = WqT_slice.T @ xqT          (lhsT=WqT chunk, rhs=xqT chunk)
  kT[dk,s]  similarly (other head's 64 partitions zeroed => K=128 strips)
  v[s,dk]   = xvT.T @ WvT_slice          (128-stride head interleave;
                                          cols 64..127 = ones for rowsum)
  scoresT[j,i] = kT_blk.T @ qT_chunk     (2 heads -> one [128,1024] psum)
  pT = exp(scoresT)                      (one ACT op per head pair)
  causal mask via affine_select on diagonal blocks (fill 0 after exp)
  xoT[dk,i] (+64 rowsum rows) = v_aug.T @ pT  (accumulated over j blocks)
  normalize: xoT *= reciprocal_approx_fast(rowsum)
  out[s,e]  = stack(xoT).T @ WoT_slice + bo/4
"""

import numpy as np
import ml_dtypes

import concourse.bass as bass
import concourse.mybir as mybir
import concourse.tile as tile
from concourse import bacc
from concourse.bass_utils import run_bass_kernel_spmd

S, B, D, H = 2048, 2, 1024, 16
DK = D // H  # 64
SCALE = 1.0 / np.sqrt(DK)
N_CORES = 8
G = N_CORES // B           # cores per batch = 4
HPC = H // G               # heads per core = 4
CPD = 256                  # cols per core = HPC * DK

F32 = mybir.dt.float32
BF16 = mybir.dt.bfloat16
NPBF = ml_dtypes.bfloat16

# biasqk (fp32, [128, 4]): cols 0:2 bqs, 2:4 bks
# biasrow (fp32, [1, 1280]): cols 0:256 bv, 256:1280 bo/G
BIASP_COLS = 4
# wpk2 layout (bf16, [128, 6144]): wk 0:2048, wq 2048:4096, wo 4096:6144
WK_OFF, WQ_OFF, WO_OFF = 0, 2048, 4096


def build_nc(mode, s=S, enable_asserts=False):
    """mode: 'causal' | 'nomask' | 'general'. Returns compiled Bass module."""
    assert s % 512 == 0
    nsc = s // 512            # 512-wide i chunks
    nsb = s // 128            # 128-wide j blocks
    nst = s // 128            # 128-row s tiles
    nd = D // 128             # contraction chunks over D

    nc = bacc.Bacc(
        "TRN2",
        target_bir_lowering=False,
        debug=False,
        enable_asserts=enable_asserts,
        num_devices=N_CORES,
    )

    xqT = nc.dram_tensor("xqT", [D, s], BF16, kind="ExternalInput")
    xkT = nc.dram_tensor("xkT", [D, s], BF16, kind="ExternalInput")
    xvT = nc.dram_tensor("xvT", [D, s], BF16, kind="ExternalInput")
    wpk1 = nc.dram_tensor("wpk1", [128, 2048], BF16, kind="ExternalInput")
    biasqk_d = nc.dram_tensor("biasqk", [128, 4], F32, kind="ExternalInput")
    biasrow_d = nc.dram_tensor("biasrow", [1, 1280], F32,
                               kind="ExternalInput")
    wpk2 = nc.dram_tensor("wpk2", [128, 6144], BF16, kind="ExternalInput")
    if mode == "general":
        maskT_d = nc.dram_tensor("maskT", [s, s], BF16, kind="ExternalInput")
    outp = nc.dram_tensor("outp", [s, D], BF16, kind="ExternalOutput")

    with tile.TileContext(nc) as tc, nc.allow_low_precision("bf16 kernel"):
        with (
            tc.tile_pool(name="const", bufs=1) as cpool,
            tc.tile_pool(name="wpool", bufs=1) as wpool,
            tc.tile_pool(name="acts", bufs=1) as apool,
        ):
            # persistent activations
            qT_sb = [apool.tile([128, s], BF16, tag=f"qT{hp}", name=f"qT{hp}")
                     for hp in range(2)]
            # kT pair tiles: partitions 0:64 = head 2hp, 64:128 = head
            # 2hp+1; score matmuls are row-tiled (two concurrent K=64
            # matmuls on different PE row groups)
            kTp_sb = [apool.tile([128, s], BF16, tag=f"kTp{hp}", name=f"kTp{hp}")
                      for hp in range(2)]
            vaug_sb = [apool.tile([128, 128 * HPC], BF16, tag=f"va{st}",
                                  name=f"va{st}") for st in range(nst)]
            stack_sb = [[apool.tile([128, 512], BF16, tag=f"st{hp}_{c}",
                                    name=f"st{hp}_{c}")
                         for c in range(nsc)] for hp in range(2)]

            # static init off the critical path: ones for the rowsum
            # columns (v-drain overwrites the dk halves), zeros for the
            # dead partition halves of kTz
            for st in range(nst):
                nc.gpsimd.memset(vaug_sb[st][:], 1.0)

            # prewarm the ACT exp table during the DMA lead-in
            zcol = cpool.tile([128, 1], F32, tag="zcol")
            nc.gpsimd.memset(zcol[:], 0.0)
            warm = cpool.tile([128, 1], BF16, tag="warm")
            nc.scalar.activation(warm[:], zcol[:],
                                 mybir.ActivationFunctionType.Exp)

            w1 = wpool.tile([128, 2048], BF16, tag="w1")
            nc.sync.dma_start(w1[:], wpk1[:])

            # ---------------- projections ----------------
            with (
                tc.tile_pool(name="xt", bufs=6) as xt_pool,
                tc.tile_pool(name="pp", bufs=8, space="PSUM") as ppool,
            ):
                # v first (its psum frees early so attention can start
                # right after q), then k, then q.
                xt0 = xt_pool.tile([128, s], BF16, tag="xt")
                nc.sync.dma_start(xt0[:], xvT[0:128, :])
                w2 = wpool.tile([128, 6144], BF16, tag="w2")
                nc.sync.dma_start(w2[:], wpk2[:])
                biasp = wpool.tile([128, 4], F32, tag="biasp")
                nc.sync.dma_start(biasp[:], biasqk_d[:])
                biasrow = wpool.tile([1, 1280], F32, tag="biasrow")
                nc.sync.dma_start(biasrow[:], biasrow_d[:])

                bvb = cpool.tile([128, CPD], F32, tag="bvb", name="bvb")
                nc.gpsimd.partition_broadcast(bvb[:], biasrow[0:1, 0:CPD])
                bo4b = cpool.tile([128, D], F32, tag="bo4b", name="bo4b")
                nc.gpsimd.partition_broadcast(bo4b[:], biasrow[0:1, CPD:CPD + D])

                vps = [ppool.tile([128, 512], F32, tag="pp", name="vps")
                       for _ in range(nst // 2)]
                # PE warm-up: dummy matmuls on already-memset tiles keep the
                # HAM activity window busy during the DMA lead-in so the
                # first real matmuls run at full clock
                for _ in range(14):
                    nc.tensor.matmul(
                        vps[nst // 2 - 1][:],
                        vaug_sb[0][:, 0:128],
                        vaug_sb[1][:],
                        start=True,
                        stop=True,
                    )
                for d in range(nd):
                    if d == 0:
                        xt = xt0
                    else:
                        xt = xt_pool.tile([128, s], BF16, tag="xt")
                        nc.sync.dma_start(xt[:], xvT[128 * d:128 * d + 128, :])
                    for st in range(nst):
                        # both 256-wide halves of a bank form ONE psum
                        # accumulation group (zero-region = whole bank)
                        nc.tensor.matmul(
                            vps[st // 2][:, 256 * (st % 2):256 * (st % 2) + 256],
                            xt[:, 128 * st:128 * st + 128],
                            w1[:, 256 * d:256 * d + 256],
                            start=(d == 0 and st % 2 == 0),
                            stop=(d == nd - 1 and st % 2 == 1),
                        )
                for st in range(nst):
                    nc.vector.tensor_add(
                        vaug_sb[st].rearrange("p (h c) -> p h c",
                                              h=HPC)[:, :, 0:64],
                        vps[st // 2][:, 256 * (st % 2):
                                     256 * (st % 2) + 256].rearrange(
                            "p (h c) -> p h c", h=HPC),
                        bvb[:].rearrange("p (h c) -> p h c", h=HPC),
                    )

                for which, w_off, b_off, scl in (
                    ("k", WK_OFF, 2, 1.0),
                    ("q", WQ_OFF, 0, SCALE),
                ):
                    ps = [[ppool.tile([128, 512], F32, tag="pp", name="pp")
                           for _ in range(nsc)] for _ in range(2)]
                    for d in range(nd):
                        xt = xt_pool.tile([128, s], BF16, tag="xt")
                        src = xkT if which == "k" else xqT
                        nc.sync.dma_start(xt[:], src[128 * d:128 * d + 128, :])
                        for hp in range(2):
                            lhs = w2[:, w_off + 256 * d + 128 * hp:
                                     w_off + 256 * d + 128 * hp + 128]
                            for sc in range(nsc):
                                nc.tensor.matmul(
                                    ps[hp][sc][:],
                                    lhs,
                                    xt[:, 512 * sc:512 * sc + 512],
                                    start=(d == 0),
                                    stop=(d == nd - 1),
                                )
                    for hp in range(2):
                        for sc in range(nsc):
                            # (psum * scale) + bias, to bf16, on DVE
                            if which == "q":
                                nc.vector.tensor_scalar(
                                    qT_sb[hp][:, 512 * sc:512 * sc + 512],
                                    ps[hp][sc][:],
                                    scl,
                                    biasp[:, b_off + hp:b_off + hp + 1],
                                    mybir.AluOpType.mult,
                                    mybir.AluOpType.add,
                                )
                            else:
                                for half in range(2):
                                    r0 = 64 * half
                                    nc.vector.tensor_scalar(
                                        kTp_sb[hp][
                                            r0:r0 + 64,
                                            512 * sc:512 * sc + 512],
                                        ps[hp][sc][r0:r0 + 64, :],
                                        scl,
                                        biasp[r0:r0 + 64,
                                              b_off + hp:b_off + hp + 1],
                                        mybir.AluOpType.mult,
                                        mybir.AluOpType.add,
                                    )

            # ---------------- attention + interleaved out-proj ----------
            with (
                tc.tile_pool(name="xo", bufs=4, space="PSUM") as xo_pool,
                tc.tile_pool(name="scp", bufs=2, space="PSUM") as sc_pool,
                tc.tile_pool(name="pt", bufs=6) as pt_pool,
                tc.tile_pool(name="mk", bufs=4) as mk_pool,
                tc.tile_pool(name="rc", bufs=2) as rc_pool,
                tc.tile_pool(name="ob", bufs=3) as ob_pool,
            ):
                def emit_outproj(c, sps=range(4)):
                    # out rows 512c..512c+512, all D cols; psum ring shared
                    # with the score tiles ([128,1024] = 2 banks each)
                    for sp in sps:
                        st = 4 * c + sp
                        op2 = sc_pool.tile([128, 1024], F32, tag="scp",
                                           name="op2")
                        for nh in range(2):
                            for hp in range(2):
                                nc.tensor.matmul(
                                    op2[:, 512 * nh:512 * nh + 512],
                                    stack_sb[hp][c][:, 128 * sp:128 * sp + 128],
                                    w2[:, WO_OFF + 1024 * hp + 512 * nh:
                                       WO_OFF + 1024 * hp + 512 * nh + 512],
                                    start=(hp == 0),
                                    stop=(hp == 1),
                                )
                        ob2 = ob_pool.tile([128, 1024], BF16, tag="ob",
                                           name="ob2")
                        nc.vector.tensor_add(ob2[:], op2[:], bo4b[:])
                        nc.sync.dma_start(outp[128 * st:128 * st + 128, :],
                                          ob2[:])

                for c in range(nsc):
                    nbj = 4 * c + 4 if mode == "causal" else nsb
                    xo = [xo_pool.tile([128, 512], F32, tag="xo", name="xo")
                          for _ in range(HPC)]
                    for bj in range(nbj):
                        if mode == "general":
                            mk = mk_pool.tile([128, 512], BF16,
                                              tag="mk", name="mk")
                            nc.sync.dma_start(
                                mk[:],
                                maskT_d[128 * bj:128 * bj + 128,
                                        512 * c:512 * c + 512],
                            )
                        # cols below f0 are fully masked (j > i): skip them
                        f0 = (max(0, 128 * bj - 512 * c)
                              if mode == "causal" else 0)
                        for hp2 in range(2):
                            scp2 = sc_pool.tile([128, 1024], F32, tag="scp",
                                                name="scp2")
                            for hh in range(2):
                                r0 = 64 * hh
                                nc.tensor.matmul(
                                    scp2[:, 512 * hh + f0:512 * hh + 512],
                                    kTp_sb[hp2][r0:r0 + 64,
                                                128 * bj:128 * bj + 128],
                                    qT_sb[hp2][r0:r0 + 64,
                                               512 * c + f0:512 * c + 512],
                                    start=True,
                                    stop=True,
                                )
                            pt2 = pt_pool.tile([128, 1024], BF16, tag="pt",
                                               name="pt2")
                            # one exp per head pair; the dead zone
                            # [512:512+f0] is never read downstream
                            nc.scalar.activation(
                                pt2[:, f0:], scp2[:, f0:],
                                mybir.ActivationFunctionType.Exp)
                            if mode == "causal" and bj >= 4 * c:
                                for hh in range(2):
                                    # keep iff i >= j:
                                    # (512c+f0+f) - (128bj+p) >= 0
                                    nc.gpsimd.affine_select(
                                        out=pt2[:, 512 * hh + f0:
                                                512 * hh + 512],
                                        in_=pt2[:, 512 * hh + f0:
                                                512 * hh + 512],
                                        compare_op=mybir.AluOpType.is_ge,
                                        fill=0.0,
                                        base=512 * c + f0 - 128 * bj,
                                        pattern=[[1, 512 - f0]],
                                        channel_multiplier=-1,
                                    )
                            if mode == "general":
                                for hh in range(2):
                                    nc.vector.tensor_mul(
                                        pt2[:, 512 * hh:512 * hh + 512],
                                        pt2[:, 512 * hh:512 * hh + 512],
                                        mk[:])
                            for hh in range(2):
                                h = 2 * hp2 + hh
                                nc.tensor.matmul(
                                    xo[h][:, f0:],
                                    vaug_sb[bj][:, 128 * h:128 * h + 128],
                                    pt2[:, 512 * hh + f0:512 * hh + 512],
                                    start=(bj == 0),
                                    stop=(bj == nbj - 1),
                                )
                        if c > 0 and 3 <= bj < 7:
                            # out-proj of the previous chunk fills the PE
                            # while this chunk's exps stream on ACT; one
                            # row-tile per bj step so the exp queue never
                            # drains
                            emit_outproj(c - 1, sps=[bj - 3])
                    # stage rowsums to SBUF first (the approx op's bit
                    # tricks need raw IEEE bits, and this frees the psum
                    # chain early), then 1/rowsum + multiply per head
                    rss = []
                    for h in range(HPC):
                        rs = rc_pool.tile([64, 512], F32, tag=f"rs{h}",
                                          name="rs")
                        nc.vector.tensor_scalar_add(rs[:], xo[h][64:128, :],
                                                    0.0)
                        rss.append(rs)
                    for h in range(HPC):
                        hp, r0 = h // 2, 64 * (h % 2)
                        rcb = rc_pool.tile([64, 512], F32, tag=f"rcb{h}",
                                           name="rcb")
                        nc.vector.reciprocal_approx_fast(rcb[:], rss[h][:])
                        nc.vector.tensor_mul(
                            stack_sb[hp][c][r0:r0 + 64, :],
                            xo[h][0:64, :],
                            rcb[:],
                        )
                emit_outproj(nsc - 1)

    nc.compile()
    return nc


_NC_CACHE = {}


def _get_nc(mode, s=S):
    key = (mode, s)
    if key not in _NC_CACHE:
        _NC_CACHE[key] = build_nc(mode, s=s)
    return _NC_CACHE[key]


def detect_mode(mask):
    m2 = np.asarray(mask).reshape(mask.shape[0], mask.shape[1])
    if m2.all():
        return "nomask"
    if np.array_equal(m2, np.tril(np.ones_like(m2))):
        return "causal"
    return "general"


def _pack_w(wT_slice, n, width):
    # [n*128, width] -> [128, n*width] with chunk d at cols width*d
    return np.ascontiguousarray(
        wT_slice.reshape(n, 128, width).transpose(1, 0, 2).reshape(128, -1)
    ).astype(NPBF)


def make_in_maps(inputs, mode, s=S):
    query = np.asarray(inputs["query"], np.float32)
    key = np.asarray(inputs["key"], np.float32)
    value = np.asarray(inputs["value"], np.float32)
    Wq = np.asarray(inputs["Wq"], np.float32)
    bq = np.asarray(inputs["bq"], np.float32)
    Wk = np.asarray(inputs["Wk"], np.float32)
    bk = np.asarray(inputs["bk"], np.float32)
    Wv = np.asarray(inputs["Wv"], np.float32)
    bv = np.asarray(inputs["bv"], np.float32)
    Wo = np.asarray(inputs["Wo"], np.float32)
    bo = np.asarray(inputs["bo"], np.float32)

    xqT = [np.ascontiguousarray(query[:, b, :].T).astype(NPBF) for b in range(B)]
    xkT = [np.ascontiguousarray(key[:, b, :].T).astype(NPBF) for b in range(B)]
    xvT = [np.ascontiguousarray(value[:, b, :].T).astype(NPBF) for b in range(B)]
    WqT, WkT, WvT, WoT = Wq.T, Wk.T, Wv.T, Wo.T
    if mode == "general":
        m2 = np.asarray(inputs["mask"]).reshape(s, s)
        maskT = np.ascontiguousarray(m2.T).astype(NPBF)

    in_maps = []
    for c in range(N_CORES):
        b, g = c // G, c % G
        cs = slice(CPD * g, CPD * g + CPD)
        biasqk = np.zeros((128, 4), np.float32)
        biasqk[:, 0:2] = (bq[cs] * SCALE).reshape(2, 128).T
        biasqk[:, 2:4] = bk[cs].reshape(2, 128).T
        biasrow = np.concatenate([bv[cs], bo / G]).reshape(1, 1280)
        biasrow = np.ascontiguousarray(biasrow, np.float32)
        wpk2 = np.concatenate([
            _pack_w(np.ascontiguousarray(WkT[:, cs]), 8, CPD),
            _pack_w(np.ascontiguousarray(WqT[:, cs]), 8, CPD),
            _pack_w(np.ascontiguousarray(WoT[cs, :]), 2, D),
        ], axis=1)
        m = {
            "xqT": xqT[b],
            "xkT": xkT[b],
            "xvT": xvT[b],
            "wpk1": _pack_w(np.ascontiguousarray(WvT[:, cs]), 8, CPD),
            "biasqk": biasqk, "biasrow": biasrow,
            "wpk2": wpk2,
        }
        if mode == "general":
            m["maskT"] = maskT
        in_maps.append(m)
    return in_maps


def run(inputs, trace=False):
    """Returns (output [S,B,D] f32, exec_time_ns or None)."""
    mode = detect_mode(np.asarray(inputs["mask"]))
    nc = _get_nc(mode)
    in_maps = make_in_maps(inputs, mode)
    res = run_bass_kernel_spmd(
        nc, in_maps, list(range(N_CORES)), trace=trace)
    out = np.empty((S, B, D), np.float32)
    for b in range(B):
        acc = res.results[G * b]["outp"].astype(np.float32)
        for g in range(1, G):
            acc = acc + res.results[G * b + g]["outp"].astype(np.float32)
        out[:, b, :] = acc
    return out, res.exec_time_ns


def kernel(**inputs):
    out, _ = run(inputs, trace=False)
    return out
